# revision 10
# baseline (speedup 1.0000x reference)
"""Trainium2 Bass kernel for nn_DenoiseNet (langevin point-cloud denoiser).

Strategy (8 NeuronCores, SPMD, zero inter-core communication):
  - Shard over B(2) x 4 contiguous N-chunks of 4096 points, each core padded
    with a 64-point halo on both sides (dependency cone grows 3 pts/step,
    4 steps -> 12 needed). Edge clipping handled exactly via per-core weight
    data (zeros on interior cores), so one program runs on all cores.
  - Feature-major layout [128 feat, (k, n) cols]. Sliding-window gather and
    scatter_add become free-dim shifted access patterns; the scatter k-sum
    and the pcl update ride matmul PSUM accumulation.
  - First score-net layer split: h0 = relu(W0g.T delta[n+off_k] + Gk[n]),
    with Gk = feat@W0[3:] + b0 + W0g.T(pcl_noisy[n+off_k] - pcl_noisy[n])
    precomputed once on device (delta-tracking keeps fp32-level accuracy
    while matmuls run f16 / float32r).
"""

import sys
import numpy as np

for _p in ("/opt/trn_rl_repo",):
    if _p not in sys.path:
        sys.path.insert(0, _p)

import concourse.bass as bass
import concourse.bacc as bacc
import concourse.tile as tile
from concourse import mybir
from concourse.bass_utils import run_bass_kernel_spmd

# ---- problem constants (hardcoded per harness contract) ----
B, N, D = 2, 16384, 3
F = 128
K = 4
OFF = [-2, -1, 0, 1]
STEPS, S0, DECAY = 4, 0.2, 0.95
CHUNK, HALO, GW = 4096, 64, 2
NP = CHUNK + 2 * HALO          # 4224 local points
NB = NP + 2 * GW               # 4228 buffer cols (with guards)
R4 = K * NP                    # 16896 (k,n) columns
N_CORES = 8

f32 = mybir.dt.float32
f16 = mybir.dt.float16
AF = mybir.ActivationFunctionType
ALU = mybir.AluOpType

_CHUNKS = [(c * 512, min(512, NP - c * 512)) for c in range((NP + 511) // 512)]
_CHUNKS_NB = [(c * 512, min(512, NB - c * 512)) for c in range((NB + 511) // 512)]



def build_program(reps=1):
    """Build the SPMD Bass/Tile program. Returns compiled Bacc module."""
    nc = bacc.Bacc("TRN2", target_bir_lowering=False, debug=False)

    din = {}
    def inp(name, shape, dt):
        din[name] = nc.dram_tensor(name, list(shape), dt, kind="ExternalInput").ap()
        return din[name]

    d_pclT = inp("pclT", (4, NB), f16)
    d_delta0 = inp("delta0", (4, NB), f16)
    d_Wf1 = inp("Wf1", (3, F), f16)
    d_bf1 = inp("bf1", (F, 1), f32)
    d_WfW = inp("WfW", (F, F), f16)
    d_bg = inp("bg", (F, 1), f32)
    d_W0g = inp("W0g", (3, F), f16)
    d_W0gn = inp("W0gn", (3, F), f16)
    d_I128 = inp("I128", (F, F), f16)
    d_Wb1 = inp("Wb1", (F, F), f16)
    d_Wb2 = inp("Wb2", (F, F), f16)
    d_bb1 = inp("bb1", (F, 1), f32)
    d_bb2 = inp("bb2", (F, 1), f32)
    d_WoS = inp("WoS", (F, 3 * STEPS), f16)
    d_I4 = inp("I4aug", (4, 3 * STEPS), f16)
    d_eL = inp("eL", (F, 3 * STEPS), f16)
    d_eLn = inp("eLn", (F, 3 * STEPS), f16)
    d_eR = inp("eR", (F, 3 * STEPS), f16)
    d_eRn = inp("eRn", (F, 3 * STEPS), f16)
    d_flagL = inp("flagL", (4, 1), f32)
    d_flagR = inp("flagR", (4, 1), f32)
    d_out = nc.dram_tensor("outT", [4, CHUNK], f16, kind="ExternalOutput").ap()

    from contextlib import ExitStack
    with tile.TileContext(nc) as tc, ExitStack() as ctx:
        cpool = ctx.enter_context(tc.tile_pool(name="const", bufs=1))
        hpool = ctx.enter_context(tc.tile_pool(name="h", bufs=4))
        tpool = ctx.enter_context(tc.tile_pool(name="tiny", bufs=2))
        ps0p = ctx.enter_context(tc.tile_pool(name="ps0", bufs=2, space="PSUM"))
        ps1p = ctx.enter_context(tc.tile_pool(name="ps1", bufs=2, space="PSUM"))
        ps2p = ctx.enter_context(tc.tile_pool(name="ps2", bufs=2, space="PSUM"))
        pspp = ctx.enter_context(tc.tile_pool(name="psP", bufs=2, space="PSUM"))

        def load(dram, shape, dt, tag):
            t = cpool.tile(list(shape), dt, tag=tag)
            nc.sync.dma_start(t[:], dram[:])
            return t

        pclT = load(d_pclT, (4, NB), f16, "pclT")
        delta_a = load(d_delta0, (4, NB), f16, "delta_a")
        delta_b = load(d_delta0, (4, NB), f16, "delta_b")
        Wf1 = load(d_Wf1, (3, F), f16, "Wf1")
        bf1 = load(d_bf1, (F, 1), f32, "bf1")
        WfW = load(d_WfW, (F, F), f16, "WfW")
        bg = load(d_bg, (F, 1), f32, "bg")
        W0g = load(d_W0g, (3, F), f16, "W0g")
        W0gn = load(d_W0gn, (3, F), f16, "W0gn")
        I128 = load(d_I128, (F, F), f16, "I128")
        Wb1 = load(d_Wb1, (F, F), f16, "Wb1")
        Wb2 = load(d_Wb2, (F, F), f16, "Wb2")
        bb1 = load(d_bb1, (F, 1), f32, "bb1")
        bb2 = load(d_bb2, (F, 1), f32, "bb2")
        WoS = load(d_WoS, (F, 3 * STEPS), f16, "WoS")
        I4 = load(d_I4, (4, 3 * STEPS), f16, "I4")
        eL = load(d_eL, (F, 3 * STEPS), f16, "eL")
        eLn = load(d_eLn, (F, 3 * STEPS), f16, "eLn")
        eR = load(d_eR, (F, 3 * STEPS), f16, "eR")
        eRn = load(d_eRn, (F, 3 * STEPS), f16, "eRn")
        flagL = load(d_flagL, (4, 1), f32, "flagL")
        flagR = load(d_flagR, (4, 1), f32, "flagR")

        Gk = cpool.tile([F, R4], f16, tag="Gk")
        h2_a = cpool.tile([F, R4], f16, tag="h2_a")
        h2_b = cpool.tile([F, R4], f16, tag="h2_b")
        A0e = cpool.tile([F, NB], f16, tag="A0e")
        G0 = cpool.tile([F, NP], f16, tag="G0")

        # ---------------- preamble: A0e, G0, Gk ----------------
        for ci, (c0, fd) in enumerate(_CHUNKS_NB):
            ps = ps0p.tile([F, 512], f32, tag="ps0")
            nc.tensor.matmul(ps[:, :fd], W0g[:, :], pclT[0:3, c0:c0 + fd],
                             start=True, stop=True)
            if ci % 2 == 0:
                nc.scalar.activation(A0e[:, c0:c0 + fd], ps[:, :fd], AF.Copy)
            else:
                nc.vector.tensor_copy(A0e[:, c0:c0 + fd], ps[:, :fd])
        for ci, (c0, fd) in enumerate(_CHUNKS):
            ps = ps1p.tile([F, 512], f32, tag="ps1")
            nc.tensor.matmul(ps[:, :fd], Wf1[:, :], pclT[0:3, GW + c0:GW + c0 + fd],
                             start=True, stop=True)
            hf = hpool.tile([F, 512], f16, tag="h0")
            nc.scalar.activation(hf[:, :fd], ps[:, :fd], AF.Relu, bias=bf1[:, :])
            ps2 = ps2p.tile([F, 512], f32, tag="ps2")
            nc.tensor.matmul(ps2[:, :fd], WfW[:, :], hf[:, :fd], start=True, stop=False)
            nc.tensor.matmul(ps2[:, :fd], W0gn[:, :], pclT[0:3, GW + c0:GW + c0 + fd],
                             start=False, stop=True)
            nc.scalar.activation(G0[:, c0:c0 + fd], ps2[:, :fd], AF.Identity, bias=bg[:, :])
        for k in range(K):
            for c0, fd in _CHUNKS:
                nc.vector.tensor_add(Gk[:, k * NP + c0:k * NP + c0 + fd],
                                     G0[:, c0:c0 + fd],
                                     A0e[:, GW + OFF[k] + c0:GW + OFF[k] + c0 + fd])

        # ---------------- langevin steps ----------------
        for rep in range(reps):
            for step in range(STEPS):
                gi = rep * STEPS + step
                d_in = delta_a if gi % 2 == 0 else delta_b
                d_out_t = delta_b if gi % 2 == 0 else delta_a
                h2 = h2_a if gi % 2 == 0 else h2_b
                final = (step == STEPS - 1) and (rep == reps - 1)
                s3 = slice(3 * step, 3 * step + 3)

                for cb, (c0, fd) in enumerate(_CHUNKS):
                    for k in range(K):
                        hcol = k * NP + c0
                        # layer0: psum = W0g.T delta_shift + I128.T Gk
                        ps0 = ps0p.tile([F, 512], f32, tag="ps0")
                        nc.tensor.matmul(ps0[:, :fd], W0g[:, :],
                                         d_in[0:3, GW + OFF[k] + c0:GW + OFF[k] + c0 + fd],
                                         start=True, stop=False)
                        nc.tensor.matmul(ps0[:, :fd], I128[:, :],
                                         Gk[:, hcol:hcol + fd], start=False, stop=True)
                        h0 = hpool.tile([F, 512], f16, tag="h0")
                        nc.scalar.activation(h0[:, :fd], ps0[:, :fd], AF.Relu)
                        # block 1
                        ps1 = ps1p.tile([F, 512], f32, tag="ps1")
                        nc.tensor.matmul(ps1[:, :fd], Wb1[:, :], h0[:, :fd],
                                         start=True, stop=True)
                        r1 = hpool.tile([F, 512], f16, tag="r1")
                        nc.vector.tensor_scalar(r1[:, :fd], ps1[:, :fd], bb1[:, :], 0.0,
                                                ALU.add, ALU.max)
                        h1 = h2[:, hcol:hcol + fd]
                        if k < 2:
                            nc.gpsimd.tensor_add(h1, h0[:, :fd], r1[:, :fd])
                        else:
                            nc.vector.tensor_add(h1, h0[:, :fd], r1[:, :fd])
                        # block 2
                        ps2 = ps2p.tile([F, 512], f32, tag="ps2")
                        nc.tensor.matmul(ps2[:, :fd], Wb2[:, :], h1,
                                         start=True, stop=True)
                        r2 = hpool.tile([F, 512], f16, tag="r2")
                        if k < 2:
                            nc.scalar.activation(r2[:, :fd], ps2[:, :fd], AF.Relu,
                                                 bias=bb2[:, :])
                        else:
                            nc.vector.tensor_scalar(r2[:, :fd], ps2[:, :fd], bb2[:, :],
                                                    0.0, ALU.add, ALU.max)
                        if k < 2:
                            nc.vector.tensor_add(h1, h1, r2[:, :fd])
                        else:
                            nc.gpsimd.tensor_add(h1, h1, r2[:, :fd])

                # scatter + update per chunk
                for cb, (c0, fd) in enumerate(_CHUNKS):
                    ps = pspp.tile([4, 512], f32, tag="psP")
                    nc.tensor.matmul(ps[0:3, :fd], I4[:, s3],
                                     d_in[0:4, GW + c0:GW + c0 + fd],
                                     start=True, stop=False)
                    for k in range(K - 1):
                        st = k * NP + c0 - OFF[k]
                        nc.tensor.matmul(ps[0:3, :fd], WoS[:, s3],
                                         h2[:, st:st + fd], start=False,
                                         stop=False)
                    if cb == 0:
                        # global-left clip fixups (eL/eLn are zero on interior cores)
                        pcol = ps[0:3, HALO:HALO + 1]
                        for col in (HALO, HALO + 1, NP + HALO):
                            nc.tensor.matmul(pcol, eL[:, s3], h2[:, col:col + 1],
                                             start=False, stop=False)
                        nc.tensor.matmul(pcol, eLn[:, s3], h2[:, 3 * NP + HALO - 1:3 * NP + HALO],
                                         start=False, stop=False)
                    if cb == len(_CHUNKS) - 1:
                        lN = HALO + CHUNK - 1
                        pN = ps[0:3, lN - c0:lN - c0 + 1]
                        nc.tensor.matmul(pN, eR[:, s3], h2[:, 3 * NP + lN:3 * NP + lN + 1],
                                         start=False, stop=False)
                        for col in (lN + 2, NP + lN + 1):
                            nc.tensor.matmul(pN, eRn[:, s3], h2[:, col:col + 1],
                                             start=False, stop=False)
                        nc.tensor.matmul(ps[0:3, lN - 1 - c0:lN - c0], eRn[:, s3],
                                         h2[:, lN + 1:lN + 2], start=False, stop=False)
                    st3 = 3 * NP + c0 - OFF[3]
                    nc.tensor.matmul(ps[0:3, :fd], WoS[:, s3],
                                     h2[:, st3:st3 + fd], start=False, stop=True)
                    dst = d_out_t[0:3, GW + c0:GW + c0 + fd]
                    if cb % 2 == 0:
                        nc.scalar.activation(dst, ps[0:3, :fd], AF.Copy)
                    else:
                        nc.vector.tensor_copy(dst, ps[0:3, :fd])

                if final:
                    nc.sync.dma_start(
                        d_out[:, :], d_out_t[0:4, GW + HALO:GW + HALO + CHUNK])
                else:
                    # mirror guards at global edges (flag=0 -> no-op on interior)
                    for flag, src_l, dst_ls in (
                        (flagL, HALO, (HALO - 2, HALO - 1)),
                        (flagR, HALO + CHUNK - 1, (HALO + CHUNK, HALO + CHUNK + 1)),
                    ):
                        for dst_l in dst_ls:
                            t = tpool.tile([4, 1], f16, tag="mir")
                            nc.vector.tensor_sub(t[0:3, :], d_out_t[0:3, GW + src_l:GW + src_l + 1],
                                                 d_out_t[0:3, GW + dst_l:GW + dst_l + 1])
                            nc.vector.tensor_scalar_mul(t[0:3, :], t[0:3, :], flag[0:3, :])
                            nc.vector.tensor_add(d_out_t[0:3, GW + dst_l:GW + dst_l + 1],
                                                 d_out_t[0:3, GW + dst_l:GW + dst_l + 1],
                                                 t[0:3, :])

    nc.compile()
    return nc


def host_prep(inputs):
    """Slice/transpose/pad inputs per core; build weight-variant constants."""
    pcl = np.asarray(inputs["pcl_noisy"], np.float32)
    Wf1 = np.asarray(inputs["Wf1"], np.float32)
    bf1 = np.asarray(inputs["bf1"], np.float32)
    Wf2 = np.asarray(inputs["Wf2"], np.float32)
    bf2 = np.asarray(inputs["bf2"], np.float32)
    W0 = np.asarray(inputs["W0"], np.float32)
    b0 = np.asarray(inputs["b0"], np.float32)
    Wb = np.asarray(inputs["Wb"], np.float32)
    bb = np.asarray(inputs["bb"], np.float32)
    Wo = np.asarray(inputs["Wo"], np.float32)
    bo = np.asarray(inputs["bo"], np.float32)

    W0g = W0[:3]
    WfW = Wf2 @ W0[3:]
    bg = bf2 @ W0[3:] + b0
    # bo multiplicity under clipped scatter
    offs = np.arange(-(K - 1) // 2, (K - 1) // 2 + 1)
    nbr = np.clip(np.arange(N)[:, None] + offs, 0, N - 1).reshape(-1)
    c_global = np.bincount(nbr, minlength=N).astype(np.float32)

    svals = [S0 * DECAY ** i for i in range(STEPS)]
    WoS = np.concatenate([s * Wo for s in svals], axis=1)          # [128, 12]
    I4 = np.zeros((4, 3 * STEPS), np.float32)
    for i, s in enumerate(svals):
        blk = np.eye(4, 3, dtype=np.float32)
        blk[3, 0:3] = s * bo
        I4[:, 3 * i:3 * i + 3] = blk

    hf = np.float16
    shared = {
        "Wf1": Wf1.astype(hf), "bf1": bf1.reshape(F, 1),
        "WfW": WfW.astype(hf), "bg": bg.reshape(F, 1),
        "W0g": W0g.astype(hf), "W0gn": (-W0g).astype(hf),
        "I128": np.eye(F, dtype=np.float32).astype(hf),
        "Wb1": Wb[0].astype(hf), "Wb2": Wb[1].astype(hf),
        "bb1": bb[0].reshape(F, 1), "bb2": bb[1].reshape(F, 1),
        "WoS": WoS.astype(hf),
        "I4aug": I4.astype(hf),
    }
    zeros_e = np.zeros((F, 3 * STEPS), np.float16)
    in_maps = []
    for core in range(N_CORES):
        b, ch = core // 4, core % 4
        g0 = ch * CHUNK - HALO
        idx = np.clip(np.arange(g0 - GW, g0 + NP + GW), 0, N - 1)
        pclT = np.empty((4, NB), np.float16)
        pclT[0:3] = pcl[b, idx].T.astype(np.float16)
        pclT[3] = 0.0
        delta0 = np.zeros((4, NB), np.float16)
        delta0[3, GW:GW + NP] = c_global[np.clip(np.arange(g0, g0 + NP), 0, N - 1)]
        isL, isR = ch == 0, ch == 3
        m = dict(shared)
        m["pclT"] = pclT
        m["delta0"] = delta0
        m["eL"] = (WoS if isL else zeros_e).astype(hf)
        m["eLn"] = (-WoS if isL else zeros_e).astype(hf)
        m["eR"] = (WoS if isR else zeros_e).astype(hf)
        m["eRn"] = (-WoS if isR else zeros_e).astype(hf)
        m["flagL"] = np.full((4, 1), 1.0 if isL else 0.0, np.float32)
        m["flagR"] = np.full((4, 1), 1.0 if isR else 0.0, np.float32)
        in_maps.append(m)
    return in_maps


_CACHED = {}


def _get_program(reps=1):
    if reps not in _CACHED:
        _CACHED[reps] = build_program(reps)
    return _CACHED[reps]


def kernel(**inputs):
    nc = _get_program(1)
    in_maps = host_prep(inputs)
    res = run_bass_kernel_spmd(nc, in_maps, list(range(N_CORES)))
    pcl = np.asarray(inputs["pcl_noisy"], np.float32)
    out = np.empty((B, N, D), np.float32)
    for core in range(N_CORES):
        b, ch = core // 4, core % 4
        sl = slice(ch * CHUNK, (ch + 1) * CHUNK)
        out[b, sl] = pcl[b, sl] + res.results[core]["outT"][0:3].T.astype(np.float32)
    return out


# revision 25
# speedup vs baseline: 391.8255x; 391.8255x over previous
"""Trainium2 Bass kernel for nn_DenoiseNet (langevin point-cloud denoiser).

Strategy (8 NeuronCores, SPMD, zero inter-core communication):
  - Shard over B(2) x 4 contiguous N-chunks of 4096 points, each core padded
    with a 64-point halo on both sides (dependency cone grows 3 pts/step,
    4 steps -> 12 needed). Global-edge clipping handled exactly via per-core
    weight data (zeros on interior cores), so one program runs on all cores.
  - Feature-major fp16 layout [128 feat, (k, n) cols]. Sliding-window gather
    and scatter_add become free-dim shifted access patterns; the scatter
    k-sum and the delta update ride matmul PSUM accumulation.
  - First score-net layer split: h0 = relu(W0g.T delta[n+off_k] + Gk[n]),
    with Gk = feat@W0[3:] + b0 + W0g.T(pcl_noisy[n+off_k] - pcl_noisy[n])
    precomputed once on device. Tracking delta (= pcl - pcl_noisy) keeps
    fp16 rounding off the large pcl values.
  - Elementwise ops run on 1024-col pairs of matmul psum tiles; ops are
    greedily load-balanced across ACT/DVE/GPSIMD.
"""

import sys
import numpy as np

for _p in ("/opt/trn_rl_repo",):
    if _p not in sys.path:
        sys.path.insert(0, _p)

import concourse.bass as bass
import concourse.bacc as bacc
import concourse.tile as tile
from concourse import mybir
from concourse.bass_utils import run_bass_kernel_spmd

# ---- problem constants (hardcoded per harness contract) ----
B, N, D = 2, 16384, 3
F = 128
K = 4
OFF = [-2, -1, 0, 1]
STEPS, S0, DECAY = 4, 0.2, 0.95
CHUNK, HALO, GW = 4096, 64, 2
NP = CHUNK + 2 * HALO          # 4224 local points
NB = NP + 2 * GW               # 4228 buffer cols (with guards)
R4 = K * NP                    # 16896 (k,n) columns
N_CORES = 8

f32 = mybir.dt.float32
f16 = mybir.dt.float16
AF = mybir.ActivationFunctionType
ALU = mybir.AluOpType

_CH512 = [(c * 512, min(512, NP - c * 512)) for c in range((NP + 511) // 512)]
_CH1024 = [(c * 1024, min(1024, NP - c * 1024)) for c in range((NP + 1023) // 1024)]
_CHNB = [(c * 512, min(512, NB - c * 512)) for c in range((NB + 511) // 512)]


def build_program(reps=1, loop_n=0):
    """Build the SPMD Bass/Tile program. Returns compiled Bacc module."""
    nc = bacc.Bacc("TRN2", target_bir_lowering=False, debug=False)

    def inp(name, shape, dt):
        return nc.dram_tensor(name, list(shape), dt, kind="ExternalInput").ap()

    d_pclT = inp("pclT", (4, NB), f16)
    d_delta0 = inp("delta0", (4, NB), f16)
    d_Wf1 = inp("Wf1", (3, F), f16)
    d_bf1 = inp("bf1", (F, 1), f32)
    d_WfW = inp("WfW", (F, F), f16)
    d_bg = inp("bg", (F, 1), f32)
    d_W0g = inp("W0g", (3, F), f16)
    d_W0gn = inp("W0gn", (3, F), f16)
    d_I128 = inp("I128", (F, F), f16)
    d_Wb1 = inp("Wb1", (F, F), f16)
    d_Wb2 = inp("Wb2", (F, F), f16)
    d_bb1 = inp("bb1", (F, 1), f32)
    d_bb2 = inp("bb2", (F, 1), f32)
    d_WoS = inp("WoS", (F, 3 * STEPS), f16)
    d_I4 = inp("I4aug", (4, 3 * STEPS), f16)
    d_eL = inp("eL", (F, 3 * STEPS), f16)
    d_eLn = inp("eLn", (F, 3 * STEPS), f16)
    d_eR = inp("eR", (F, 3 * STEPS), f16)
    d_eRn = inp("eRn", (F, 3 * STEPS), f16)
    d_flagL = inp("flagL", (4, 1), f32)
    d_flagR = inp("flagR", (4, 1), f32)
    d_out = nc.dram_tensor("outT", [4, CHUNK], f16, kind="ExternalOutput").ap()

    from contextlib import ExitStack
    with tile.TileContext(nc) as tc, ExitStack() as ctx:
        cpool = ctx.enter_context(tc.tile_pool(name="const", bufs=1))
        hpool = ctx.enter_context(tc.tile_pool(name="h", bufs=4))
        tpool = ctx.enter_context(tc.tile_pool(name="tiny", bufs=2))
        psp = ctx.enter_context(tc.tile_pool(name="ps", bufs=6, space="PSUM"))
        pspp = ctx.enter_context(tc.tile_pool(name="psP", bufs=2, space="PSUM"))
        h0pool = ctx.enter_context(tc.tile_pool(name="h0p", bufs=36))

        def load(dram, shape, dt, tag):
            t = cpool.tile(list(shape), dt, tag=tag)
            nc.sync.dma_start(t[:], dram[:])
            return t

        pclT = load(d_pclT, (4, NB), f16, "pclT")
        delta_a = load(d_delta0, (4, NB), f16, "delta_a")
        delta_b = load(d_delta0, (4, NB), f16, "delta_b")
        Wf1 = load(d_Wf1, (3, F), f16, "Wf1")
        bf1 = load(d_bf1, (F, 1), f32, "bf1")
        WfW = load(d_WfW, (F, F), f16, "WfW")
        bg = load(d_bg, (F, 1), f32, "bg")
        W0g = load(d_W0g, (3, F), f16, "W0g")
        W0gn = load(d_W0gn, (3, F), f16, "W0gn")
        I128 = load(d_I128, (F, F), f16, "I128")
        Wb1 = load(d_Wb1, (F, F), f16, "Wb1")
        Wb2 = load(d_Wb2, (F, F), f16, "Wb2")
        bb1 = load(d_bb1, (F, 1), f32, "bb1")
        bb2 = load(d_bb2, (F, 1), f32, "bb2")
        WoS = load(d_WoS, (F, 3 * STEPS), f16, "WoS")
        I4 = load(d_I4, (4, 3 * STEPS), f16, "I4")
        eL = load(d_eL, (F, 3 * STEPS), f16, "eL")
        eLn = load(d_eLn, (F, 3 * STEPS), f16, "eLn")
        eR = load(d_eR, (F, 3 * STEPS), f16, "eR")
        eRn = load(d_eRn, (F, 3 * STEPS), f16, "eRn")
        flagL = load(d_flagL, (4, 1), f32, "flagL")
        flagR = load(d_flagR, (4, 1), f32, "flagR")

        Gk = cpool.tile([F, R4], f16, tag="Gk")
        h2_a = cpool.tile([F, R4], f16, tag="h2_a")
        h2_b = cpool.tile([F, R4], f16, tag="h2_b")
        A0e = cpool.tile([F, NB], f16, tag="A0e")
        G0 = cpool.tile([F, NP], f16, tag="G0")

        # greedy engine balancer for elementwise work
        load_ns = {"ACT": 0.0, "DVE": 0.0, "GP": 0.0}

        def pick(cands):
            eng, cost, fn = min(cands, key=lambda c: load_ns[c[0]] + c[1])
            load_ns[eng] += cost
            fn()

        def relu_op(dst, src, fd, bias=None):
            # psum -> sbuf relu, optional per-partition bias
            def on_act():
                nc.scalar.activation(dst, src, AF.Relu,
                                     bias=(bias[:, :] if bias is not None else 0.0))
            def on_dve():
                if bias is not None:
                    nc.vector.tensor_scalar(dst, src, bias[:, :], 0.0, ALU.add, ALU.max)
                else:
                    nc.vector.tensor_scalar_max(dst, src, 0.0)
            pick([("ACT", (fd + 172) * 0.833 + 16, on_act),
                  ("DVE", (fd + 60) * 1.042 + 15, on_dve)])

        def copy_op(dst, src, fd):
            def on_act():
                nc.scalar.activation(dst, src, AF.Copy)
            def on_dve():
                nc.vector.tensor_copy(dst, src)
            pick([("ACT", (fd + 172) * 0.833 + 16, on_act),
                  ("DVE", (fd + 60) * 1.042 + 15, on_dve)])

        def add_op(dst, a, b, fd):
            def on_dve():
                nc.vector.tensor_add(dst, a, b)
            def on_gp():
                nc.gpsimd.tensor_add(dst, a, b)
            pick([("DVE", (fd / 2 + 52) * 1.042 + 15, on_dve),
                  ("GP", fd * 1.9, on_gp)])

        # one column at the k=2/k=3 boundary is read (as cone garbage) by the
        # interleaved scatter before any tile writes it on step 0
        nc.vector.memset(h2_a[:, 3 * NP - 1:3 * NP], 0.0)
        nc.vector.memset(h2_b[:, 3 * NP - 1:3 * NP], 0.0)

        # ---------------- preamble: A0e, G0, Gk ----------------
        for ci, (c0, fd) in enumerate(_CHNB):
            ps = psp.tile([F, 512], f32, tag="ps")
            nc.tensor.matmul(ps[:, :fd], W0g[:, :], pclT[0:3, c0:c0 + fd],
                             start=True, stop=True)
            copy_op(A0e[:, c0:c0 + fd], ps[:, :fd], fd)
        for ci, (c0, fd) in enumerate(_CH512):
            ps = psp.tile([F, 512], f32, tag="ps")
            nc.tensor.matmul(ps[:, :fd], Wf1[:, :], pclT[0:3, GW + c0:GW + c0 + fd],
                             start=True, stop=True)
            hf = hpool.tile([F, 1024], f16, tag="h0")
            nc.scalar.activation(hf[:, :fd], ps[:, :fd], AF.Relu, bias=bf1[:, :])
            ps2 = psp.tile([F, 512], f32, tag="ps")
            nc.tensor.matmul(ps2[:, :fd], WfW[:, :], hf[:, :fd], start=True, stop=False)
            nc.tensor.matmul(ps2[:, :fd], W0gn[:, :], pclT[0:3, GW + c0:GW + c0 + fd],
                             start=False, stop=True)
            nc.scalar.activation(G0[:, c0:c0 + fd], ps2[:, :fd], AF.Identity, bias=bg[:, :])
        for k in range(K):
            for c0, fd in _CH512:
                add_op(Gk[:, k * NP + c0:k * NP + c0 + fd], G0[:, c0:c0 + fd],
                       A0e[:, GW + OFF[k] + c0:GW + OFF[k] + c0 + fd], fd)

        # ---------------- langevin steps ----------------
        def emit_rep(final_rep):
            for step in range(STEPS):
                d_in = delta_a if step % 2 == 0 else delta_b
                d_out_t = delta_b if step % 2 == 0 else delta_a
                h2 = h2_a if step % 2 == 0 else h2_b
                final = (step == STEPS - 1) and final_rep
                s3 = slice(3 * step, 3 * step + 3)

                def emit_passA(cb):
                    c0, fd = _CH512[cb]
                    for k in range(K):
                        hcol = k * NP + c0
                        ps = psp.tile([F, 512], f32, tag="ps")
                        nc.tensor.matmul(
                            ps[:, :fd], W0g[:, :],
                            d_in[0:3, GW + OFF[k] + c0:GW + OFF[k] + c0 + fd],
                            start=True, stop=False)
                        nc.tensor.matmul(ps[:, :fd], I128[:, :],
                                         Gk[:, hcol:hcol + fd], start=False, stop=True)
                        h0 = h0pool.tile([F, 512], f16, tag="h0")
                        relu_op(h0[:, :fd], ps[:, :fd], fd)
                        h0s[(k, cb)] = h0

                def emit_passB(cb):
                    c0, fd = _CH512[cb]
                    for k in range(K):
                        hcol = k * NP + c0
                        h0 = h0s[(k, cb)]
                        ps = psp.tile([F, 512], f32, tag="ps")
                        nc.tensor.matmul(ps[:, :fd], Wb1[:, :], h0[:, :fd],
                                         start=True, stop=True)
                        r1 = hpool.tile([F, 512], f16, tag="r1")
                        relu_op(r1[:, :fd], ps[:, :fd], fd, bias=bb1)
                        add_op(h2[:, hcol:hcol + fd], h0[:, :fd], r1[:, :fd], fd)

                def emit_passC(cb):
                    c0, fd = _CH512[cb]
                    for k in range(K):
                        hcol = k * NP + c0
                        ps = psp.tile([F, 512], f32, tag="ps")
                        nc.tensor.matmul(ps[:, :fd], Wb2[:, :],
                                         h2[:, hcol:hcol + fd], start=True, stop=True)
                        r2 = hpool.tile([F, 512], f16, tag="r2")
                        relu_op(r2[:, :fd], ps[:, :fd], fd, bias=bb2)
                        add_op(h2[:, hcol:hcol + fd], h2[:, hcol:hcol + fd],
                               r2[:, :fd], fd)

                def mirror_fix(flag, src_l, dst_ls):
                    # mirror guards at global edges (flag=0 -> no-op on interior)
                    for dst_l in dst_ls:
                        t = tpool.tile([4, 1], f16, tag="mir")
                        nc.vector.tensor_sub(t[0:3, :],
                                             d_out_t[0:3, GW + src_l:GW + src_l + 1],
                                             d_out_t[0:3, GW + dst_l:GW + dst_l + 1])
                        nc.vector.tensor_scalar_mul(t[0:3, :], t[0:3, :], flag[0:3, :])
                        nc.vector.tensor_add(d_out_t[0:3, GW + dst_l:GW + dst_l + 1],
                                             d_out_t[0:3, GW + dst_l:GW + dst_l + 1],
                                             t[0:3, :])

                def emit_scatter(cb):
                    c0, fd = _CH512[cb]
                    ps = pspp.tile([4, 512], f32, tag="psP")
                    for k in range(K):
                        st = k * NP + c0 - OFF[k]
                        nc.tensor.matmul(ps[0:3, :fd], WoS[:, s3],
                                         h2[:, st:st + fd],
                                         start=(k == 0), stop=False)
                    if cb == 0:
                        pcol = ps[0:3, HALO:HALO + 1]
                        for col in (HALO, HALO + 1, NP + HALO):
                            nc.tensor.matmul(pcol, eL[:, s3], h2[:, col:col + 1],
                                             start=False, stop=False)
                        nc.tensor.matmul(pcol, eLn[:, s3],
                                         h2[:, 3 * NP + HALO - 1:3 * NP + HALO],
                                         start=False, stop=False)
                    if cb == len(_CH512) - 1:
                        lN = HALO + CHUNK - 1
                        pN = ps[0:3, lN - c0:lN - c0 + 1]
                        nc.tensor.matmul(pN, eR[:, s3], h2[:, 3 * NP + lN:3 * NP + lN + 1],
                                         start=False, stop=False)
                        for col in (lN + 2, NP + lN + 1):
                            nc.tensor.matmul(pN, eRn[:, s3], h2[:, col:col + 1],
                                             start=False, stop=False)
                        nc.tensor.matmul(ps[0:3, lN - 1 - c0:lN - c0], eRn[:, s3],
                                         h2[:, lN + 1:lN + 2], start=False, stop=False)
                    nc.tensor.matmul(ps[0:3, :fd], I4[:, s3],
                                     d_in[0:4, GW + c0:GW + c0 + fd],
                                     start=False, stop=True)
                    copy_op(d_out_t[0:3, GW + c0:GW + c0 + fd], ps[0:3, :fd], fd)

                h0s = {}
                nblk = len(_CH512)
                for cb in range(nblk + 3):
                    if cb < nblk:
                        emit_passA(cb)
                    if 1 <= cb + 1 - 1 and 0 <= cb - 1 < nblk:
                        emit_passB(cb - 1)
                    if 0 <= cb - 2 < nblk:
                        emit_passC(cb - 2)
                    if 0 <= cb - 3 < nblk:
                        emit_scatter(cb - 3)

                if final:
                    nc.sync.dma_start(
                        d_out[:, :], d_out_t[0:4, GW + HALO:GW + HALO + CHUNK])
                else:
                    mirror_fix(flagL, HALO, (HALO - 2, HALO - 1))
                    mirror_fix(flagR, HALO + CHUNK - 1, (HALO + CHUNK,))

        if loop_n:
            with tc.For_i(0, loop_n, 1):
                emit_rep(False)
            emit_rep(True)
        else:
            for rep in range(reps):
                emit_rep(rep == reps - 1)

    nc.compile()
    return nc


def host_prep(inputs):
    """Slice/transpose/pad inputs per core; build weight-variant constants."""
    pcl = np.asarray(inputs["pcl_noisy"], np.float32)
    Wf1 = np.asarray(inputs["Wf1"], np.float32)
    bf1 = np.asarray(inputs["bf1"], np.float32)
    Wf2 = np.asarray(inputs["Wf2"], np.float32)
    bf2 = np.asarray(inputs["bf2"], np.float32)
    W0 = np.asarray(inputs["W0"], np.float32)
    b0 = np.asarray(inputs["b0"], np.float32)
    Wb = np.asarray(inputs["Wb"], np.float32)
    bb = np.asarray(inputs["bb"], np.float32)
    Wo = np.asarray(inputs["Wo"], np.float32)
    bo = np.asarray(inputs["bo"], np.float32)

    W0g = W0[:3]
    WfW = Wf2 @ W0[3:]
    bg = bf2 @ W0[3:] + b0
    offs = np.arange(-(K - 1) // 2, (K - 1) // 2 + 1)
    nbr = np.clip(np.arange(N)[:, None] + offs, 0, N - 1).reshape(-1)
    c_global = np.bincount(nbr, minlength=N).astype(np.float32)

    svals = [S0 * DECAY ** i for i in range(STEPS)]
    WoS = np.concatenate([s * Wo for s in svals], axis=1)          # [128, 12]
    I4 = np.zeros((4, 3 * STEPS), np.float32)
    for i, s in enumerate(svals):
        blk = np.eye(4, 3, dtype=np.float32)
        blk[3, 0:3] = s * bo
        I4[:, 3 * i:3 * i + 3] = blk

    hf = np.float16
    shared = {
        "Wf1": Wf1.astype(hf), "bf1": bf1.reshape(F, 1),
        "WfW": WfW.astype(hf), "bg": bg.reshape(F, 1),
        "W0g": W0g.astype(hf), "W0gn": (-W0g).astype(hf),
        "I128": np.eye(F, dtype=np.float32).astype(hf),
        "Wb1": Wb[0].astype(hf), "Wb2": Wb[1].astype(hf),
        "bb1": bb[0].reshape(F, 1), "bb2": bb[1].reshape(F, 1),
        "WoS": WoS.astype(hf),
        "I4aug": I4.astype(hf),
    }
    zeros_e = np.zeros((F, 3 * STEPS), np.float16)
    in_maps = []
    for core in range(N_CORES):
        b, ch = core // 4, core % 4
        g0 = ch * CHUNK - HALO
        idx = np.clip(np.arange(g0 - GW, g0 + NP + GW), 0, N - 1)
        pclT = np.empty((4, NB), np.float16)
        pclT[0:3] = pcl[b, idx].T.astype(np.float16)
        pclT[3] = 0.0
        delta0 = np.zeros((4, NB), np.float16)
        delta0[3, GW:GW + NP] = c_global[np.clip(np.arange(g0, g0 + NP), 0, N - 1)]
        isL, isR = ch == 0, ch == 3
        m = dict(shared)
        m["pclT"] = pclT
        m["delta0"] = delta0
        m["eL"] = (WoS.astype(hf) if isL else zeros_e)
        m["eLn"] = ((-WoS).astype(hf) if isL else zeros_e)
        m["eR"] = (WoS.astype(hf) if isR else zeros_e)
        m["eRn"] = ((-WoS).astype(hf) if isR else zeros_e)
        m["flagL"] = np.full((4, 1), 1.0 if isL else 0.0, np.float32)
        m["flagR"] = np.full((4, 1), 1.0 if isR else 0.0, np.float32)
        in_maps.append(m)
    return in_maps


_CACHED = {}


def _get_program(reps=1):
    if reps not in _CACHED:
        _CACHED[reps] = build_program(reps)
    return _CACHED[reps]


def kernel(**inputs):
    nc = _get_program(1)
    in_maps = host_prep(inputs)
    res = run_bass_kernel_spmd(nc, in_maps, list(range(N_CORES)))
    pcl = np.asarray(inputs["pcl_noisy"], np.float32)
    out = np.empty((B, N, D), np.float32)
    for core in range(N_CORES):
        b, ch = core // 4, core % 4
        sl = slice(ch * CHUNK, (ch + 1) * CHUNK)
        out[b, sl] = pcl[b, sl] + res.results[core]["outT"][0:3].T.astype(np.float32)
    return out
